# revision 19
# baseline (speedup 1.0000x reference)
"""GCN message-passing kernel for 8 trn2 NeuronCores (Bass/Tile) — v2.

Math (reference):
  x1 = relu(segsum(feat) @ W1 + b1)
  x2 = relu(segsum(x1) @ W2 + b2)
  out = relu(x2 @ W3 + b3)
where segsum(X)[i] = sum_{e: dst[e]=i} X[src[e]].

Reorder: segsum(X) @ W == segsum(X @ W):
  h0 = feat @ W1            (token-major matmul, bf16)
  x1 = relu(segsum(h0)+b1)  (gather h0 rows by src, segment-sum by dst)
  h1 = x1 @ W2
  x2 = relu(segsum(h1)+b2)
  out = relu(x2 @ W3 + b3)

v2 changes vs v1 (971us baseline):
 - h0/h1 stored fp8e4m3: AllGather + gather DMA bytes halved; S tiles fp8
   (integer counts, exact). Aggregation matmuls fp8 DoubleRow (2 chunk pairs
   per PE pass). Phase-A / W2 / W3 matmuls stay bf16 (fp8 there breaks the
   2e-2 rel-err budget; measured 2.05e-2 in numpy sim).
 - AllGather(h0) split by column halves: AG(half0) overlaps phase-A compute
   of half1; AG(half1) overlaps L1 aggregation of half0.
 - AllGather(h1) split by row halves (blocks 0-9 / 10-19) so the first AG
   overlaps the tail of L1/W2; gather row ids for layer 2 are remapped to the
   concatenated [a;b] layout on the host.
 - S selection tiles loaded into SBUF once and reused across both column
   halves and both layers (same edge structure).
 - Gathers spread across 2 SWDGE queues.
"""
import numpy as np
import ml_dtypes

import concourse.bass as bass
import concourse.bacc as bacc
import concourse.tile as tile
import concourse.mybir as mybir
from concourse import bass_utils

bf16 = ml_dtypes.bfloat16
f8e4 = ml_dtypes.float8_e4m3

NCORES = 8
N_NODES = 20000
N_EDGES = 200000
D_IN = 1433
KF = 1536           # padded feature dim (12 x 128)
H1 = 1024           # padded hidden1 (real 1000)
H2 = 512            # padded hidden2 (real 500)
DO = 7
R = N_NODES // NCORES          # 2500 rows per core
TB = [128] * 19 + [68]         # token/dst blocks per core (sum = 2500)
NB = len(TB)
TB_OFF = np.concatenate([[0], np.cumsum(TB)]).astype(int)
NBA = 10                        # blocks in first h1 AllGather row-split
RS_A = int(TB_OFF[NBA])         # 1280 rows
RS_B = R - RS_A                 # 1220 rows


def _chunk_groups(kb, nq=4):
    """Split kb chunks into up to nq contiguous groups of near-equal size."""
    nq = min(nq, kb)
    base = kb // nq
    rem = kb % nq
    sizes = [base + (1 if i < rem else 0) for i in range(nq)]
    out = []
    a = 0
    for s in sizes:
        out.append((a, a + s))
        a += s
    return out


def _wrap_idx(ids, npad):
    """int16 wrapped gather-idx layout: [16, npad/16] tiled to 128 partitions."""
    pad = np.zeros(npad, np.int64)
    pad[: len(ids)] = ids
    wrapped = pad.reshape(-1, 16).T.astype(np.int16)  # [16, npad/16]
    return np.tile(wrapped, (8, 1))                   # [128, npad/16]


def _host_prep(features, src, dst, W1, b1, W2, b2, W3, b3):
    """Build per-core staged arrays (all sharding/sorting/padding on host)."""
    feat = np.asarray(features, np.float32)
    src = np.asarray(src).astype(np.int64)
    dst = np.asarray(dst).astype(np.int64)

    featT = np.zeros((KF, N_NODES), np.float32)
    featT[:D_IN, :] = feat.T
    featT = featT.astype(bf16)

    W1p = np.zeros((KF, H1), np.float32)
    W1p[:D_IN, : W1.shape[1]] = W1
    W1p = W1p.astype(bf16)
    W2p = np.zeros((H1, H2), np.float32)
    W2p[: W2.shape[0], : W2.shape[1]] = W2
    W2p = W2p.astype(bf16)
    # W3 host-swizzled to [128, 4*DO]: W3sw[p, j*DO:(j+1)*DO] = W3p[j*128+p, :]
    W3p = np.zeros((H2, DO), np.float32)
    W3p[: W3.shape[0], :] = W3
    W3sw = np.zeros((128, (H2 // 128) * DO), np.float32)
    for j in range(H2 // 128):
        W3sw[:, j * DO : (j + 1) * DO] = W3p[j * 128 : (j + 1) * 128, :]
    W3sw = W3sw.astype(bf16)

    b1p = np.zeros((1, H1), np.float32)
    b1p[0, : b1.shape[0]] = b1
    b2p = np.zeros((1, H2), np.float32)
    b2p[0, : b2.shape[0]] = b2
    b3p = np.zeros((1, DO), np.float32)
    b3p[0, : b3.shape[0]] = b3
    has_bias = bool(np.any(b1p) or np.any(b2p) or np.any(b3p))

    ident = np.eye(128, dtype=bf16)

    # ---- edge prep: partition by dst owner, sort by dst, chunk per dst-block
    owner = dst // R
    edge_src = [[] for _ in range(NCORES)]
    for c in range(NCORES):
        sel = np.nonzero(owner == c)[0]
        d_loc = dst[sel] - c * R
        order = np.argsort(d_loc, kind="stable")
        sel = sel[order]
        d_loc = d_loc[order]
        s_glob = src[sel]
        blk_of = np.searchsorted(TB_OFF[1:], d_loc, side="right")
        per_blk = []
        for b in range(NB):
            m = blk_of == b
            uniq, inv = np.unique(s_glob[m], return_inverse=True)
            per_blk.append((uniq, inv, d_loc[m] - TB_OFF[b]))
        edge_src[c] = per_blk

    # split each block's unique srcs by source-row region (r < RS_A within the
    # owning core -> region a). Region-wise EVEN chunk counts, uniform across
    # cores (SPMD + DoubleRow pairs). Region a reads h0all_a (row-split AG),
    # region b reads h0all_b.
    def _region_of(ids):
        return (ids % R) < RS_A

    per_cb = [[None] * NB for _ in range(NCORES)]
    Ka_blk = [2] * NB
    Kb_blk = [2] * NB
    for c in range(NCORES):
        for b in range(NB):
            uniq, inv, d_l = edge_src[c][b]
            in_a = _region_of(uniq)
            ua, ub = uniq[in_a], uniq[~in_a]
            per_cb[c][b] = (ua, ub, uniq, inv, d_l)
            ka = -(-len(ua) // 128)
            kb_ = -(-len(ub) // 128)
            Ka_blk[b] = max(Ka_blk[b], ka + (ka & 1))
            Kb_blk[b] = max(Kb_blk[b], kb_ + (kb_ & 1))
    K_blk = [Ka_blk[b] + Kb_blk[b] for b in range(NB)]
    TC = sum(K_blk)
    CI0 = np.concatenate([[0], np.cumsum(K_blk)]).astype(int)
    OFF16 = np.concatenate([[0], np.cumsum([k * 8 for k in K_blk])]).astype(int)
    TOT16 = int(OFF16[-1])

    # h0 row-split remap: node (c, r): r < RS_A -> h0all_a row c*RS_A + r,
    # else h0all_b row c*RS_B + (r - RS_A)
    def h0_remap(ids):
        c = ids // R
        r = ids - c * R
        return np.where(r < RS_A, c * RS_A + r, c * RS_B + (r - RS_A))

    idx1_per_core = []
    s_sw_per_core = []
    for c in range(NCORES):
        i1 = np.zeros((128, TOT16), np.int16)
        s_f32 = np.zeros((TC, 128, 128), np.float32)
        for b in range(NB):
            ua, ub, uniq, inv, d_l = per_cb[c][b]
            # slot order: region-a srcs in chunks [0, Ka), region-b srcs in
            # chunks [Ka, Ka+Kb). Map each edge's uniq-index to its slot.
            ka, kb_ = Ka_blk[b], Kb_blk[b]
            slot_of_uniq = np.zeros(len(uniq), np.int64)
            in_a = _region_of(uniq)
            slot_of_uniq[in_a] = np.arange(len(ua))
            slot_of_uniq[~in_a] = ka * 128 + np.arange(len(ub))
            slot = slot_of_uniq[inv]
            np.add.at(s_f32, (CI0[b] + slot // 128, slot % 128, d_l), 1.0)
            i1[:, OFF16[b] : OFF16[b] + ka * 8] = _wrap_idx(h0_remap(ua), ka * 128)
            i1[:, OFF16[b] + ka * 8 : OFF16[b + 1]] = _wrap_idx(h0_remap(ub), kb_ * 128)
        idx1_per_core.append(i1)
        s_sw_per_core.append(
            np.ascontiguousarray(s_f32.astype(f8e4).transpose(1, 0, 2))
        )

    static = dict(K_blk=K_blk, Ka_blk=Ka_blk, Kb_blk=Kb_blk, TC=TC, CI0=CI0,
                  OFF16=OFF16, TOT16=TOT16, has_bias=has_bias)
    shared = dict(W1p=W1p, W2p=W2p, W3sw=W3sw, b1p=b1p.astype(bf16),
                  b2p=b2p.astype(bf16), b3p=b3p, ident=ident)
    in_maps = []
    for c in range(NCORES):
        in_maps.append(
            dict(
                featT=np.ascontiguousarray(featT[:, c * R : (c + 1) * R]),
                idx1=idx1_per_core[c],
                s_sw=s_sw_per_core[c],
                **shared,
            )
        )
    return static, in_maps


def _build_program(static):
    K_blk, TC, CI0, OFF16, TOT16 = (
        static["K_blk"], static["TC"], static["CI0"], static["OFF16"], static["TOT16"],
    )
    Ka_blk, Kb_blk = static["Ka_blk"], static["Kb_blk"]
    has_bias = static["has_bias"]
    f32 = mybir.dt.float32
    b16 = mybir.dt.bfloat16
    e4 = mybir.dt.float8e4
    i16 = mybir.dt.int16
    DR = mybir.MatmulPerfMode.DoubleRow

    nc = bacc.Bacc(
        "TRN2", target_bir_lowering=False, debug=False,
        enable_asserts=False, num_devices=NCORES, num_swdge_queues=4,
    )

    featT_d = nc.dram_tensor("featT", [KF, R], b16, kind="ExternalInput")
    W1_d = nc.dram_tensor("W1p", [KF, H1], b16, kind="ExternalInput")
    W2_d = nc.dram_tensor("W2p", [H1, H2], b16, kind="ExternalInput")
    W3_d = nc.dram_tensor("W3sw", [128, (H2 // 128) * DO], b16, kind="ExternalInput")
    b1_d = nc.dram_tensor("b1p", [1, H1], b16, kind="ExternalInput")
    b2_d = nc.dram_tensor("b2p", [1, H2], b16, kind="ExternalInput")
    b3_d = nc.dram_tensor("b3p", [1, DO], f32, kind="ExternalInput")
    id_d = nc.dram_tensor("ident", [128, 128], b16, kind="ExternalInput")
    idx1_d = nc.dram_tensor("idx1", [128, TOT16], i16, kind="ExternalInput")
    ssw_d = nc.dram_tensor("s_sw", [128, TC, 128], e4, kind="ExternalInput")
    out_d = nc.dram_tensor("out", [R, DO], f32, kind="ExternalOutput")

    kmax = max(K_blk)

    with tile.TileContext(nc) as tc:
        qsem = [nc.alloc_semaphore(name=f"swdge_q{q}") for q in range(4)]
        with (
            tc.tile_pool(name="const", bufs=1) as constp,
            tc.tile_pool(name="w2p", bufs=H1 // 128) as w2p,
            tc.tile_pool(name="w3p", bufs=1) as w3p,
            tc.tile_pool(name="dram", bufs=1, space="DRAM") as dram,
        ):
            # ---- constants
            idx1_sb = constp.tile([128, TOT16], i16, tag="idx1")
            nc.sync.dma_start(idx1_sb[:], idx1_d.ap())
            st_all = constp.tile([128, TC, 128], e4, tag="st_all")
            nc.sync.dma_start(st_all[:], ssw_d.ap())
            ident = constp.tile([128, 128], b16, tag="ident")
            nc.sync.dma_start(ident[:], id_d.ap())
            ones1 = constp.tile([1, 128], b16, tag="ones1")
            nc.vector.memset(ones1[:], 1.0)
            b1_sb = constp.tile([1, H1], b16, tag="b1")
            nc.sync.dma_start(b1_sb[:], b1_d.ap())
            b2_sb = constp.tile([1, H2], b16, tag="b2")
            nc.sync.dma_start(b2_sb[:], b2_d.ap())
            b3_sb = constp.tile([1, DO], f32, tag="b3")
            nc.sync.dma_start(b3_sb[:], b3_d.ap())

            # ---- DRAM scratch (fp8 halves of h0; h1 row-split for the AG)
            h0_in_a = dram.tile([RS_A, H1], e4, name="h0ina", tag="h0ina")
            h0_in_b = dram.tile([RS_B, H1], e4, name="h0inb", tag="h0inb")
            h0all_a = dram.tile([NCORES * RS_A, H1], e4, name="h0alla", tag="h0alla", addr_space="Shared")
            h0all_b = dram.tile([NCORES * RS_B, H1], e4, name="h0allb", tag="h0allb", addr_space="Shared")
            h1_in_a = dram.tile([RS_A, H2], e4, name="h1ina", tag="h1ina")
            h1_in_b = dram.tile([RS_B, H2], e4, name="h1inb", tag="h1inb")
            h1all_a = dram.tile([NCORES * RS_A, H2], e4, name="h1alla", tag="h1alla", addr_space="Shared")
            h1all_b = dram.tile([NCORES * RS_B, H2], e4, name="h1allb", tag="h1allb", addr_space="Shared")

            # ================= Phase A: h0 = featT.T @ W1, column halves
            w2c = []
            for j in range(H1 // 128):
                wt = w2p.tile([128, H2], b16, name=f"w2c{j}", tag="w2c")
                nc.sync.dma_start(wt[:], W2_d.ap()[j * 128 : (j + 1) * 128, :])
                w2c.append(wt)
            w3t = w3p.tile([128, (H2 // 128) * DO], b16, tag="w3")
            nc.sync.dma_start(w3t[:], W3_d.ap())

            with (
                tc.tile_pool(name="featp", bufs=KF // 128) as featp,
                tc.tile_pool(name="w1p", bufs=KF // 128) as w1p,
            ):
                featc = []
                w1c = []
                for k in range(KF // 128):
                    ft = featp.tile([128, R], b16, name=f"featc{k}", tag="featc")
                    nc.sync.dma_start(ft[:], featT_d.ap()[k * 128 : (k + 1) * 128, :])
                    featc.append(ft)
                    wt = w1p.tile([128, H1], b16, name=f"w1c{k}", tag="w1c")
                    nc.sync.dma_start(wt[:], W1_d.ap()[k * 128 : (k + 1) * 128, :])
                    w1c.append(wt)
                with (
                    tc.tile_pool(name="h0out", bufs=4) as h0outp,
                    tc.tile_pool(name="psA", bufs=6, space="PSUM") as psA,
                ):
                    nk = KF // 128
                    with nc.named_scope("phaseA"):
                        for t in range(NB):
                            sl = slice(TB_OFF[t], TB_OFF[t + 1])
                            tb = TB[t]
                            ps = [psA.tile([128, 512], f32, name=f"psA_{j}_{t}", tag="psA")
                                  for j in range(2)]
                            for k in range(nk):
                                for j in range(2):
                                    nc.tensor.matmul(
                                        ps[j][:tb, :],
                                        featc[k][:, sl],
                                        w1c[k][:, j * 512 : (j + 1) * 512],
                                        start=(k == 0),
                                        stop=(k == nk - 1),
                                    )
                            o = h0outp.tile([128, H1], e4, name=f"h0o_{t}", tag="h0o")
                            for j in range(2):
                                nc.vector.tensor_copy(
                                    o[:tb, j * 512 : (j + 1) * 512], ps[j][:tb, :])
                            if t < NBA:
                                nc.sync.dma_start(h0_in_a[:][sl, :], o[:tb, :])
                            else:
                                sl_b = slice(TB_OFF[t] - RS_A, TB_OFF[t + 1] - RS_A)
                                nc.sync.dma_start(h0_in_b[:][sl_b, :], o[:tb, :])
                            if t == NBA - 1:
                                nc.gpsimd.collective_compute(
                                    "AllGather", mybir.AluOpType.bypass,
                                    replica_groups=[list(range(NCORES))],
                                    ins=[h0_in_a.opt()], outs=[h0all_a.opt()],
                                )
            # ================= L1 aggregation per half + transpose; W2 -> h1
            with (
                tc.tile_pool(name="gout", bufs=7) as goutp,
                tc.tile_pool(name="x1h", bufs=4) as x1hp,
                tc.tile_pool(name="x1T", bufs=H1 // 128) as x1Tp,
                tc.tile_pool(name="h1o", bufs=3) as h1op,
                tc.tile_pool(name="psAgg", bufs=4, space="PSUM") as psAgg,
                tc.tile_pool(name="psTr", bufs=2, space="PSUM") as psTr,
                tc.tile_pool(name="psH1", bufs=2, space="PSUM") as psH1,
            ):
                x1T = [x1Tp.tile([128, R], b16, name=f"x1T{_j}", tag="x1T") for _j in range(H1 // 128)]

                nj = H1 // 128
                LAG = 2
                PRE = 6
                x1hs = {}
                gs = {}

                def _emit_ga(b, g):
                    ka = Ka_blk[b]
                    for qi, (a0, a1) in enumerate(_chunk_groups(ka, 2)):
                        nc.gpsimd.dma_gather(
                            g[:, a0:a1, :], h0all_a[:],
                            idx1_sb[:, OFF16[b] + a0 * 8 : OFF16[b] + a1 * 8],
                            num_idxs=(a1 - a0) * 128,
                            num_idxs_reg=(a1 - a0) * 128,
                            elem_size=H1, single_packet=False,
                            queue_num=qi,
                        )

                with nc.named_scope("phaseC"):
                    # region-a gathers for the first blocks run during AG1b
                    for b in range(PRE):
                        g = goutp.tile([128, kmax, H1], e4, name=f"gout_{b}", tag="gout")
                        gs[b] = g
                        _emit_ga(b, g)
                    nc.gpsimd.collective_compute(
                        "AllGather", mybir.AluOpType.bypass,
                        replica_groups=[list(range(NCORES))],
                        ins=[h0_in_b.opt()], outs=[h0all_b.opt()],
                    )
                    for bb in range(NB + LAG):
                      if bb < NB:
                        b = bb
                        kb = K_blk[b]
                        sl = slice(TB_OFF[b], TB_OFF[b + 1])
                        if b in gs:
                            g = gs.pop(b)
                        else:
                            g = goutp.tile([128, kmax, H1], e4, name=f"gout_{b}", tag="gout")
                            _emit_ga(b, g)
                        ka = Ka_blk[b]
                        for qi, (a0, a1) in enumerate(_chunk_groups(kb - ka, 2)):
                            nc.gpsimd.dma_gather(
                                g[:, ka + a0 : ka + a1, :], h0all_b[:],
                                idx1_sb[:, OFF16[b] + (ka + a0) * 8 : OFF16[b] + (ka + a1) * 8],
                                num_idxs=(a1 - a0) * 128,
                                num_idxs_reg=(a1 - a0) * 128,
                                elem_size=H1, single_packet=False,
                                queue_num=2 + qi,
                            )
                        aggs = [psAgg.tile([128, 512], f32, name=f"agg_{j}_{b}", tag="agg")
                                for j in range(2)]
                        pair_off = [2 * i for i in range(ka // 2)] + \
                                   [ka + 2 * i for i in range((kb - ka) // 2)]
                        for pi, po_ in enumerate(pair_off):
                            for j in range(2):
                                nc.tensor.matmul(
                                    aggs[j][:],
                                    st_all[:, CI0[b] + po_ : CI0[b] + po_ + 2, :],
                                    g[:, po_ : po_ + 2, j * 512 : (j + 1) * 512],
                                    start=(pi == 0),
                                    stop=(pi == len(pair_off) - 1) and not has_bias,
                                    perf_mode=DR,
                                )
                        if has_bias:
                            for j in range(2):
                                nc.tensor.matmul(
                                    aggs[j][:], ones1[:],
                                    b1_sb[:, j * 512 : (j + 1) * 512],
                                    start=False, stop=True, skip_group_check=True,
                                )
                        x1h = x1hp.tile([128, H1], b16, name=f"x1h_{b}", tag="x1h")
                        x1hs[b] = x1h
                        for j in range(2):
                            nc.vector.tensor_scalar_max(
                                x1h[:, j * 512 : (j + 1) * 512], aggs[j][:], 0.0)
                      if bb >= LAG:
                        b = bb - LAG
                        tb = TB[b]
                        sl = slice(TB_OFF[b], TB_OFF[b + 1])
                        x1h = x1hs.pop(b)
                        for jj in range(8):
                            tr = psTr.tile([128, 128], b16, name=f"tr_{b}_{jj}", tag="tr")
                            nc.tensor.transpose(
                                tr[:, :tb],
                                x1h[:tb, jj * 128 : (jj + 1) * 128],
                                ident[:tb, :tb],
                            )
                            nc.vector.tensor_copy(x1T[jj][:, sl], tr[:, :tb])
                        ph = psH1.tile([128, H2], f32, name=f"psh1_{b}", tag="psh1")
                        for j in range(nj):
                            nc.tensor.matmul(
                                ph[:tb, :], x1T[j][:, sl], w2c[j][:],
                                start=(j == 0), stop=(j == nj - 1),
                            )
                        ho = h1op.tile([128, H2], e4, name=f"h1o_{b}", tag="h1o")
                        nc.vector.tensor_copy(ho[:tb, :], ph[:tb, :])
                        if b < NBA:
                            nc.sync.dma_start(h1_in_a[:][sl, :], ho[:tb, :])
                        else:
                            sl_b2 = slice(TB_OFF[b] - RS_A, TB_OFF[b + 1] - RS_A)
                            nc.sync.dma_start(h1_in_b[:][sl_b2, :], ho[:tb, :])
                        if b == NBA - 1:
                            nc.gpsimd.collective_compute(
                                "AllGather", mybir.AluOpType.bypass,
                                replica_groups=[list(range(NCORES))],
                                ins=[h1_in_a.opt()], outs=[h1all_a.opt()],
                            )
                    nc.gpsimd.collective_compute(
                        "AllGather", mybir.AluOpType.bypass,
                        replica_groups=[list(range(NCORES))],
                        ins=[h1_in_b.opt()], outs=[h1all_b.opt()],
                    )

            # ================= L2 aggregation + relu; W3 -> out
            with (
                tc.tile_pool(name="gout2", bufs=8) as goutp2,
                tc.tile_pool(name="x2p", bufs=6) as x2p,
                tc.tile_pool(name="x2T", bufs=H2 // 128) as x2Tp,
                tc.tile_pool(name="outp", bufs=3) as outp,
                tc.tile_pool(name="psAgg2", bufs=4, space="PSUM") as psAgg2,
                tc.tile_pool(name="psTr2", bufs=2, space="PSUM") as psTr2,
                tc.tile_pool(name="psO", bufs=2, space="PSUM") as psO,
            ):
                x2T = [x2Tp.tile([128, R], b16, name=f"x2T{_j}", tag="x2T") for _j in range(H2 // 128)]

                with nc.named_scope("phaseF"):
                    njj = H2 // 128
                    LAG = 3
                    x2bs = {}
                    for bb in range(NB + LAG):
                      if bb < NB:
                        b = bb
                        kb = K_blk[b]
                        g = goutp2.tile([128, kmax, H2], e4, name=f"gout2_{b}", tag="gout2")
                        ka2 = Ka_blk[b]
                        for qi, (a0, a1) in enumerate(_chunk_groups(ka2, 2)):
                            nc.gpsimd.dma_gather(
                                g[:, a0:a1, :], h1all_a[:],
                                idx1_sb[:, OFF16[b] + a0 * 8 : OFF16[b] + a1 * 8],
                                num_idxs=(a1 - a0) * 128,
                                num_idxs_reg=(a1 - a0) * 128,
                                elem_size=H2, single_packet=False,
                                queue_num=qi,
                            )
                        for qi, (a0, a1) in enumerate(_chunk_groups(kb - ka2, 2)):
                            nc.gpsimd.dma_gather(
                                g[:, ka2 + a0 : ka2 + a1, :], h1all_b[:],
                                idx1_sb[:, OFF16[b] + (ka2 + a0) * 8 : OFF16[b] + (ka2 + a1) * 8],
                                num_idxs=(a1 - a0) * 128,
                                num_idxs_reg=(a1 - a0) * 128,
                                elem_size=H2, single_packet=False,
                                queue_num=2 + qi,
                            )
                        agg = psAgg2.tile([128, H2], f32, name=f"agg2_{b}", tag="agg2")
                        for i in range(kb // 2):
                            nc.tensor.matmul(
                                agg[:],
                                st_all[:, CI0[b] + 2 * i : CI0[b] + 2 * i + 2, :],
                                g[:, 2 * i : 2 * i + 2, :],
                                start=(i == 0),
                                stop=(i == kb // 2 - 1) and not has_bias,
                                perf_mode=DR,
                            )
                        if has_bias:
                            nc.tensor.matmul(
                                agg[:], ones1[:], b2_sb[:],
                                start=False, stop=True, skip_group_check=True,
                            )
                        x2b = x2p.tile([128, H2], b16, name=f"x2_{b}", tag="x2")
                        x2bs[b] = x2b
                        nc.vector.tensor_scalar_max(x2b[:], agg[:], 0.0)
                      if bb >= LAG:
                        b = bb - LAG
                        tb = TB[b]
                        sl = slice(TB_OFF[b], TB_OFF[b + 1])
                        x2b = x2bs.pop(b)
                        for jj in range(njj):
                            tr = psTr2.tile([128, 128], b16, name=f"tr2_{b}_{jj}", tag="tr2")
                            nc.tensor.transpose(
                                tr[:, :tb],
                                x2b[:tb, jj * 128 : (jj + 1) * 128],
                                ident[:tb, :tb],
                            )
                            nc.vector.tensor_copy(x2T[jj][:, sl], tr[:, :tb])
                        po = psO.tile([128, DO], f32, name=f"pso_{b}", tag="pso")
                        for jj in range(njj):
                            nc.tensor.matmul(
                                po[:tb, :], x2T[jj][:, sl],
                                w3t[:, jj * DO : (jj + 1) * DO],
                                start=(jj == 0),
                                stop=(jj == njj - 1) and not has_bias,
                            )
                        if has_bias:
                            nc.tensor.matmul(
                                po[:tb, :], ones1[:, :tb], b3_sb[:],
                                start=False, stop=True, skip_group_check=True,
                            )
                        oo = outp.tile([128, DO], f32, name=f"oo_{b}", tag="oo")
                        nc.vector.tensor_scalar_max(oo[:tb, :], po[:tb, :], 0.0)
                        nc.sync.dma_start(out_d.ap()[sl, :], oo[:tb, :])

    nc.compile()
    return nc


def kernel_with_results(features, src, dst, W1, b1, W2, b2, W3, b3, trace=False):
    static, in_maps = _host_prep(features, src, dst, W1, b1, W2, b2, W3, b3)
    nc = _build_program(static)
    res = bass_utils.run_bass_kernel_spmd(
        nc, in_maps, core_ids=list(range(NCORES)), trace=trace
    )
    out = np.concatenate([res.results[c]["out"] for c in range(NCORES)], axis=0)
    return out.astype(np.float32), res


def kernel(features, src, dst, W1, b1, W2, b2, W3, b3):
    out, _ = kernel_with_results(features, src, dst, W1, b1, W2, b2, W3, b3)
    return out


# revision 20
# speedup vs baseline: 1.0193x; 1.0193x over previous
"""GCN message-passing kernel for 8 trn2 NeuronCores (Bass/Tile) — v2.

Math (reference):
  x1 = relu(segsum(feat) @ W1 + b1)
  x2 = relu(segsum(x1) @ W2 + b2)
  out = relu(x2 @ W3 + b3)
where segsum(X)[i] = sum_{e: dst[e]=i} X[src[e]].

Reorder: segsum(X) @ W == segsum(X @ W):
  h0 = feat @ W1            (token-major matmul, bf16)
  x1 = relu(segsum(h0)+b1)  (gather h0 rows by src, segment-sum by dst)
  h1 = x1 @ W2
  x2 = relu(segsum(h1)+b2)
  out = relu(x2 @ W3 + b3)

v2 changes vs v1 (971us baseline):
 - h0/h1 stored fp8e4m3: AllGather + gather DMA bytes halved; S tiles fp8
   (integer counts, exact). Aggregation matmuls fp8 DoubleRow (2 chunk pairs
   per PE pass). Phase-A / W2 / W3 matmuls stay bf16 (fp8 there breaks the
   2e-2 rel-err budget; measured 2.05e-2 in numpy sim).
 - AllGather(h0) split by column halves: AG(half0) overlaps phase-A compute
   of half1; AG(half1) overlaps L1 aggregation of half0.
 - AllGather(h1) split by row halves (blocks 0-9 / 10-19) so the first AG
   overlaps the tail of L1/W2; gather row ids for layer 2 are remapped to the
   concatenated [a;b] layout on the host.
 - S selection tiles loaded into SBUF once and reused across both column
   halves and both layers (same edge structure).
 - Gathers spread across 2 SWDGE queues.
"""
import numpy as np
import ml_dtypes

import concourse.bass as bass
import concourse.bacc as bacc
import concourse.tile as tile
import concourse.mybir as mybir
from concourse import bass_utils

bf16 = ml_dtypes.bfloat16
f8e4 = ml_dtypes.float8_e4m3

NCORES = 8
N_NODES = 20000
N_EDGES = 200000
D_IN = 1433
KF = 1536           # padded feature dim (12 x 128)
H1 = 1024           # padded hidden1 (real 1000)
H2 = 512            # padded hidden2 (real 500)
DO = 7
R = N_NODES // NCORES          # 2500 rows per core
TB = [128] * 19 + [68]         # token/dst blocks per core (sum = 2500)
NB = len(TB)
TB_OFF = np.concatenate([[0], np.cumsum(TB)]).astype(int)
NBA = 10                        # blocks in first h1 AllGather row-split
RS_A = int(TB_OFF[NBA])         # 1280 rows
RS_B = R - RS_A                 # 1220 rows


def _chunk_groups(kb, nq=4):
    """Split kb chunks into up to nq contiguous groups of near-equal size."""
    nq = min(nq, kb)
    base = kb // nq
    rem = kb % nq
    sizes = [base + (1 if i < rem else 0) for i in range(nq)]
    out = []
    a = 0
    for s in sizes:
        out.append((a, a + s))
        a += s
    return out


def _wrap_idx(ids, npad):
    """int16 wrapped gather-idx layout: [16, npad/16] tiled to 128 partitions."""
    pad = np.zeros(npad, np.int64)
    pad[: len(ids)] = ids
    wrapped = pad.reshape(-1, 16).T.astype(np.int16)  # [16, npad/16]
    return np.tile(wrapped, (8, 1))                   # [128, npad/16]


def _host_prep(features, src, dst, W1, b1, W2, b2, W3, b3):
    """Build per-core staged arrays (all sharding/sorting/padding on host)."""
    feat = np.asarray(features, np.float32)
    src = np.asarray(src).astype(np.int64)
    dst = np.asarray(dst).astype(np.int64)

    featT = np.zeros((KF, N_NODES), np.float32)
    featT[:D_IN, :] = feat.T
    featT = featT.astype(bf16)

    W1p = np.zeros((KF, H1), np.float32)
    W1p[:D_IN, : W1.shape[1]] = W1
    W1p = W1p.astype(bf16)
    W2p = np.zeros((H1, H2), np.float32)
    W2p[: W2.shape[0], : W2.shape[1]] = W2
    W2p = W2p.astype(bf16)
    # W3 host-swizzled to [128, 4*DO]: W3sw[p, j*DO:(j+1)*DO] = W3p[j*128+p, :]
    W3p = np.zeros((H2, DO), np.float32)
    W3p[: W3.shape[0], :] = W3
    W3sw = np.zeros((128, (H2 // 128) * DO), np.float32)
    for j in range(H2 // 128):
        W3sw[:, j * DO : (j + 1) * DO] = W3p[j * 128 : (j + 1) * 128, :]
    W3sw = W3sw.astype(bf16)

    b1p = np.zeros((1, H1), np.float32)
    b1p[0, : b1.shape[0]] = b1
    b2p = np.zeros((1, H2), np.float32)
    b2p[0, : b2.shape[0]] = b2
    b3p = np.zeros((1, DO), np.float32)
    b3p[0, : b3.shape[0]] = b3
    has_bias = bool(np.any(b1p) or np.any(b2p) or np.any(b3p))

    ident = np.eye(128, dtype=bf16)

    # ---- edge prep: partition by dst owner, sort by dst, chunk per dst-block
    owner = dst // R
    edge_src = [[] for _ in range(NCORES)]
    for c in range(NCORES):
        sel = np.nonzero(owner == c)[0]
        d_loc = dst[sel] - c * R
        order = np.argsort(d_loc, kind="stable")
        sel = sel[order]
        d_loc = d_loc[order]
        s_glob = src[sel]
        blk_of = np.searchsorted(TB_OFF[1:], d_loc, side="right")
        per_blk = []
        for b in range(NB):
            m = blk_of == b
            uniq, inv = np.unique(s_glob[m], return_inverse=True)
            per_blk.append((uniq, inv, d_loc[m] - TB_OFF[b]))
        edge_src[c] = per_blk

    # split each block's unique srcs by source-row region (r < RS_A within the
    # owning core -> region a). Region-wise EVEN chunk counts, uniform across
    # cores (SPMD + DoubleRow pairs). Region a reads h0all_a (row-split AG),
    # region b reads h0all_b.
    def _region_of(ids):
        return (ids % R) < RS_A

    per_cb = [[None] * NB for _ in range(NCORES)]
    Ka_blk = [2] * NB
    Kb_blk = [2] * NB
    for c in range(NCORES):
        for b in range(NB):
            uniq, inv, d_l = edge_src[c][b]
            in_a = _region_of(uniq)
            ua, ub = uniq[in_a], uniq[~in_a]
            per_cb[c][b] = (ua, ub, uniq, inv, d_l)
            ka = -(-len(ua) // 128)
            kb_ = -(-len(ub) // 128)
            Ka_blk[b] = max(Ka_blk[b], ka + (ka & 1))
            Kb_blk[b] = max(Kb_blk[b], kb_ + (kb_ & 1))
    K_blk = [Ka_blk[b] + Kb_blk[b] for b in range(NB)]
    TC = sum(K_blk)
    CI0 = np.concatenate([[0], np.cumsum(K_blk)]).astype(int)
    OFF16 = np.concatenate([[0], np.cumsum([k * 8 for k in K_blk])]).astype(int)
    TOT16 = int(OFF16[-1])

    # h0 row-split remap: node (c, r): r < RS_A -> h0all_a row c*RS_A + r,
    # else h0all_b row c*RS_B + (r - RS_A)
    def h0_remap(ids):
        c = ids // R
        r = ids - c * R
        return np.where(r < RS_A, c * RS_A + r, c * RS_B + (r - RS_A))

    idx1_per_core = []
    s_sw_per_core = []
    for c in range(NCORES):
        i1 = np.zeros((128, TOT16), np.int16)
        s_f32 = np.zeros((TC, 128, 128), np.float32)
        for b in range(NB):
            ua, ub, uniq, inv, d_l = per_cb[c][b]
            # slot order: region-a srcs in chunks [0, Ka), region-b srcs in
            # chunks [Ka, Ka+Kb). Map each edge's uniq-index to its slot.
            ka, kb_ = Ka_blk[b], Kb_blk[b]
            slot_of_uniq = np.zeros(len(uniq), np.int64)
            in_a = _region_of(uniq)
            slot_of_uniq[in_a] = np.arange(len(ua))
            slot_of_uniq[~in_a] = ka * 128 + np.arange(len(ub))
            slot = slot_of_uniq[inv]
            np.add.at(s_f32, (CI0[b] + slot // 128, slot % 128, d_l), 1.0)
            i1[:, OFF16[b] : OFF16[b] + ka * 8] = _wrap_idx(h0_remap(ua), ka * 128)
            i1[:, OFF16[b] + ka * 8 : OFF16[b + 1]] = _wrap_idx(h0_remap(ub), kb_ * 128)
        idx1_per_core.append(i1)
        s_sw_per_core.append(
            np.ascontiguousarray(s_f32.astype(f8e4).transpose(1, 0, 2))
        )

    static = dict(K_blk=K_blk, Ka_blk=Ka_blk, Kb_blk=Kb_blk, TC=TC, CI0=CI0,
                  OFF16=OFF16, TOT16=TOT16, has_bias=has_bias)
    shared = dict(W1p=W1p, W2p=W2p, W3sw=W3sw, b1p=b1p.astype(bf16),
                  b2p=b2p.astype(bf16), b3p=b3p, ident=ident)
    in_maps = []
    for c in range(NCORES):
        in_maps.append(
            dict(
                featT=np.ascontiguousarray(featT[:, c * R : (c + 1) * R]),
                idx1=idx1_per_core[c],
                s_sw=s_sw_per_core[c],
                **shared,
            )
        )
    return static, in_maps


def _build_program(static):
    K_blk, TC, CI0, OFF16, TOT16 = (
        static["K_blk"], static["TC"], static["CI0"], static["OFF16"], static["TOT16"],
    )
    Ka_blk, Kb_blk = static["Ka_blk"], static["Kb_blk"]
    has_bias = static["has_bias"]
    f32 = mybir.dt.float32
    b16 = mybir.dt.bfloat16
    e4 = mybir.dt.float8e4
    i16 = mybir.dt.int16
    DR = mybir.MatmulPerfMode.DoubleRow

    nc = bacc.Bacc(
        "TRN2", target_bir_lowering=False, debug=False,
        enable_asserts=False, num_devices=NCORES, num_swdge_queues=4,
    )

    featT_d = nc.dram_tensor("featT", [KF, R], b16, kind="ExternalInput")
    W1_d = nc.dram_tensor("W1p", [KF, H1], b16, kind="ExternalInput")
    W2_d = nc.dram_tensor("W2p", [H1, H2], b16, kind="ExternalInput")
    W3_d = nc.dram_tensor("W3sw", [128, (H2 // 128) * DO], b16, kind="ExternalInput")
    b1_d = nc.dram_tensor("b1p", [1, H1], b16, kind="ExternalInput")
    b2_d = nc.dram_tensor("b2p", [1, H2], b16, kind="ExternalInput")
    b3_d = nc.dram_tensor("b3p", [1, DO], f32, kind="ExternalInput")
    id_d = nc.dram_tensor("ident", [128, 128], b16, kind="ExternalInput")
    idx1_d = nc.dram_tensor("idx1", [128, TOT16], i16, kind="ExternalInput")
    ssw_d = nc.dram_tensor("s_sw", [128, TC, 128], e4, kind="ExternalInput")
    out_d = nc.dram_tensor("out", [R, DO], f32, kind="ExternalOutput")

    kmax = max(K_blk)

    with tile.TileContext(nc) as tc:
        qsem = [nc.alloc_semaphore(name=f"swdge_q{q}") for q in range(4)]
        with (
            tc.tile_pool(name="const", bufs=1) as constp,
            tc.tile_pool(name="w2p", bufs=H1 // 128) as w2p,
            tc.tile_pool(name="w3p", bufs=1) as w3p,
            tc.tile_pool(name="dram", bufs=1, space="DRAM") as dram,
        ):
            # ---- constants
            idx1_sb = constp.tile([128, TOT16], i16, tag="idx1")
            nc.sync.dma_start(idx1_sb[:], idx1_d.ap())
            st_all = constp.tile([128, TC, 128], e4, tag="st_all")
            nc.sync.dma_start(st_all[:], ssw_d.ap())
            ident = constp.tile([128, 128], b16, tag="ident")
            nc.sync.dma_start(ident[:], id_d.ap())
            ones1 = constp.tile([1, 128], b16, tag="ones1")
            nc.vector.memset(ones1[:], 1.0)
            b1_sb = constp.tile([1, H1], b16, tag="b1")
            nc.sync.dma_start(b1_sb[:], b1_d.ap())
            b2_sb = constp.tile([1, H2], b16, tag="b2")
            nc.sync.dma_start(b2_sb[:], b2_d.ap())
            b3_sb = constp.tile([1, DO], f32, tag="b3")
            nc.sync.dma_start(b3_sb[:], b3_d.ap())

            # ---- DRAM scratch (fp8 halves of h0; h1 row-split for the AG)
            h0_in_a = dram.tile([RS_A, H1], e4, name="h0ina", tag="h0ina")
            h0_in_b = dram.tile([RS_B, H1], e4, name="h0inb", tag="h0inb")
            h0all_a = dram.tile([NCORES * RS_A, H1], e4, name="h0alla", tag="h0alla", addr_space="Shared")
            h0all_b = dram.tile([NCORES * RS_B, H1], e4, name="h0allb", tag="h0allb", addr_space="Shared")
            h1_in_a = dram.tile([RS_A, H2], e4, name="h1ina", tag="h1ina")
            h1_in_b = dram.tile([RS_B, H2], e4, name="h1inb", tag="h1inb")
            h1all_a = dram.tile([NCORES * RS_A, H2], e4, name="h1alla", tag="h1alla", addr_space="Shared")
            h1all_b = dram.tile([NCORES * RS_B, H2], e4, name="h1allb", tag="h1allb", addr_space="Shared")

            # ================= Phase A: h0 = featT.T @ W1, column halves
            w2c = []
            for j in range(H1 // 128):
                wt = w2p.tile([128, H2], b16, name=f"w2c{j}", tag="w2c")
                nc.sync.dma_start(wt[:], W2_d.ap()[j * 128 : (j + 1) * 128, :])
                w2c.append(wt)
            w3t = w3p.tile([128, (H2 // 128) * DO], b16, tag="w3")
            nc.sync.dma_start(w3t[:], W3_d.ap())

            with (
                tc.tile_pool(name="featp", bufs=KF // 128) as featp,
                tc.tile_pool(name="w1p", bufs=KF // 128) as w1p,
            ):
                featc = []
                w1c = []
                for k in range(KF // 128):
                    ft = featp.tile([128, R], b16, name=f"featc{k}", tag="featc")
                    nc.sync.dma_start(ft[:], featT_d.ap()[k * 128 : (k + 1) * 128, :])
                    featc.append(ft)
                    wt = w1p.tile([128, H1], b16, name=f"w1c{k}", tag="w1c")
                    nc.sync.dma_start(wt[:], W1_d.ap()[k * 128 : (k + 1) * 128, :])
                    w1c.append(wt)
                with (
                    tc.tile_pool(name="h0out", bufs=4) as h0outp,
                    tc.tile_pool(name="psA", bufs=6, space="PSUM") as psA,
                ):
                    nk = KF // 128
                    with nc.named_scope("phaseA"):
                        for t in range(NB):
                            sl = slice(TB_OFF[t], TB_OFF[t + 1])
                            tb = TB[t]
                            ps = [psA.tile([128, 512], f32, name=f"psA_{j}_{t}", tag="psA")
                                  for j in range(2)]
                            for k in range(nk):
                                for j in range(2):
                                    nc.tensor.matmul(
                                        ps[j][:tb, :],
                                        featc[k][:, sl],
                                        w1c[k][:, j * 512 : (j + 1) * 512],
                                        start=(k == 0),
                                        stop=(k == nk - 1),
                                    )
                            o = h0outp.tile([128, H1], e4, name=f"h0o_{t}", tag="h0o")
                            for j in range(2):
                                nc.vector.tensor_copy(
                                    o[:tb, j * 512 : (j + 1) * 512], ps[j][:tb, :])
                            if t < NBA:
                                nc.sync.dma_start(h0_in_a[:][sl, :], o[:tb, :])
                            else:
                                sl_b = slice(TB_OFF[t] - RS_A, TB_OFF[t + 1] - RS_A)
                                nc.sync.dma_start(h0_in_b[:][sl_b, :], o[:tb, :])
                            if t == NBA - 1:
                                nc.gpsimd.collective_compute(
                                    "AllGather", mybir.AluOpType.bypass,
                                    replica_groups=[list(range(NCORES))],
                                    ins=[h0_in_a.opt()], outs=[h0all_a.opt()],
                                )
            # ================= L1 aggregation per half + transpose; W2 -> h1
            with (
                tc.tile_pool(name="gout", bufs=7) as goutp,
                tc.tile_pool(name="x1h", bufs=4) as x1hp,
                tc.tile_pool(name="x1T", bufs=H1 // 128) as x1Tp,
                tc.tile_pool(name="h1o", bufs=3) as h1op,
                tc.tile_pool(name="psAgg", bufs=4, space="PSUM") as psAgg,
                tc.tile_pool(name="psTr", bufs=2, space="PSUM") as psTr,
                tc.tile_pool(name="psH1", bufs=2, space="PSUM") as psH1,
            ):
                x1T = [x1Tp.tile([128, R], b16, name=f"x1T{_j}", tag="x1T") for _j in range(H1 // 128)]

                nj = H1 // 128
                LAG = 2
                PRE = 3
                x1hs = {}
                gs = {}

                def _emit_ga(b, g):
                    ka = Ka_blk[b]
                    for qi, (a0, a1) in enumerate(_chunk_groups(ka, 2)):
                        nc.gpsimd.dma_gather(
                            g[:, a0:a1, :], h0all_a[:],
                            idx1_sb[:, OFF16[b] + a0 * 8 : OFF16[b] + a1 * 8],
                            num_idxs=(a1 - a0) * 128,
                            num_idxs_reg=(a1 - a0) * 128,
                            elem_size=H1, single_packet=False,
                            queue_num=qi,
                        )

                with nc.named_scope("phaseC"):
                    # region-a gathers for the first blocks run during AG1b
                    for b in range(PRE):
                        g = goutp.tile([128, kmax, H1], e4, name=f"gout_{b}", tag="gout")
                        gs[b] = g
                        _emit_ga(b, g)
                    nc.gpsimd.collective_compute(
                        "AllGather", mybir.AluOpType.bypass,
                        replica_groups=[list(range(NCORES))],
                        ins=[h0_in_b.opt()], outs=[h0all_b.opt()],
                    )
                    for bb in range(NB + LAG):
                      if bb < NB:
                        b = bb
                        kb = K_blk[b]
                        sl = slice(TB_OFF[b], TB_OFF[b + 1])
                        if b in gs:
                            g = gs.pop(b)
                        else:
                            g = goutp.tile([128, kmax, H1], e4, name=f"gout_{b}", tag="gout")
                            _emit_ga(b, g)
                        ka = Ka_blk[b]
                        for qi, (a0, a1) in enumerate(_chunk_groups(kb - ka, 2)):
                            nc.gpsimd.dma_gather(
                                g[:, ka + a0 : ka + a1, :], h0all_b[:],
                                idx1_sb[:, OFF16[b] + (ka + a0) * 8 : OFF16[b] + (ka + a1) * 8],
                                num_idxs=(a1 - a0) * 128,
                                num_idxs_reg=(a1 - a0) * 128,
                                elem_size=H1, single_packet=False,
                                queue_num=2 + qi,
                            )
                        aggs = [psAgg.tile([128, 512], f32, name=f"agg_{j}_{b}", tag="agg")
                                for j in range(2)]
                        pair_off = [2 * i for i in range(ka // 2)] + \
                                   [ka + 2 * i for i in range((kb - ka) // 2)]
                        for pi, po_ in enumerate(pair_off):
                            for j in range(2):
                                nc.tensor.matmul(
                                    aggs[j][:],
                                    st_all[:, CI0[b] + po_ : CI0[b] + po_ + 2, :],
                                    g[:, po_ : po_ + 2, j * 512 : (j + 1) * 512],
                                    start=(pi == 0),
                                    stop=(pi == len(pair_off) - 1) and not has_bias,
                                    perf_mode=DR,
                                )
                        if has_bias:
                            for j in range(2):
                                nc.tensor.matmul(
                                    aggs[j][:], ones1[:],
                                    b1_sb[:, j * 512 : (j + 1) * 512],
                                    start=False, stop=True, skip_group_check=True,
                                )
                        x1h = x1hp.tile([128, H1], b16, name=f"x1h_{b}", tag="x1h")
                        x1hs[b] = x1h
                        for j in range(2):
                            nc.vector.tensor_scalar_max(
                                x1h[:, j * 512 : (j + 1) * 512], aggs[j][:], 0.0)
                      if bb >= LAG:
                        b = bb - LAG
                        tb = TB[b]
                        sl = slice(TB_OFF[b], TB_OFF[b + 1])
                        x1h = x1hs.pop(b)
                        for jj in range(8):
                            tr = psTr.tile([128, 128], b16, name=f"tr_{b}_{jj}", tag="tr")
                            nc.tensor.transpose(
                                tr[:, :tb],
                                x1h[:tb, jj * 128 : (jj + 1) * 128],
                                ident[:tb, :tb],
                            )
                            nc.vector.tensor_copy(x1T[jj][:, sl], tr[:, :tb])
                        ph = psH1.tile([128, H2], f32, name=f"psh1_{b}", tag="psh1")
                        for j in range(nj):
                            nc.tensor.matmul(
                                ph[:tb, :], x1T[j][:, sl], w2c[j][:],
                                start=(j == 0), stop=(j == nj - 1),
                            )
                        ho = h1op.tile([128, H2], e4, name=f"h1o_{b}", tag="h1o")
                        nc.vector.tensor_copy(ho[:tb, :], ph[:tb, :])
                        if b < NBA:
                            nc.sync.dma_start(h1_in_a[:][sl, :], ho[:tb, :])
                        else:
                            sl_b2 = slice(TB_OFF[b] - RS_A, TB_OFF[b + 1] - RS_A)
                            nc.sync.dma_start(h1_in_b[:][sl_b2, :], ho[:tb, :])
                        if b == NBA - 1:
                            nc.gpsimd.collective_compute(
                                "AllGather", mybir.AluOpType.bypass,
                                replica_groups=[list(range(NCORES))],
                                ins=[h1_in_a.opt()], outs=[h1all_a.opt()],
                            )
                    nc.gpsimd.collective_compute(
                        "AllGather", mybir.AluOpType.bypass,
                        replica_groups=[list(range(NCORES))],
                        ins=[h1_in_b.opt()], outs=[h1all_b.opt()],
                    )

            # ================= L2 aggregation + relu; W3 -> out
            with (
                tc.tile_pool(name="gout2", bufs=8) as goutp2,
                tc.tile_pool(name="x2p", bufs=6) as x2p,
                tc.tile_pool(name="x2T", bufs=H2 // 128) as x2Tp,
                tc.tile_pool(name="outp", bufs=3) as outp,
                tc.tile_pool(name="psAgg2", bufs=4, space="PSUM") as psAgg2,
                tc.tile_pool(name="psTr2", bufs=2, space="PSUM") as psTr2,
                tc.tile_pool(name="psO", bufs=2, space="PSUM") as psO,
            ):
                x2T = [x2Tp.tile([128, R], b16, name=f"x2T{_j}", tag="x2T") for _j in range(H2 // 128)]

                with nc.named_scope("phaseF"):
                    njj = H2 // 128
                    LAG = 3
                    x2bs = {}
                    for bb in range(NB + LAG):
                      if bb < NB:
                        b = bb
                        kb = K_blk[b]
                        g = goutp2.tile([128, kmax, H2], e4, name=f"gout2_{b}", tag="gout2")
                        ka2 = Ka_blk[b]
                        for qi, (a0, a1) in enumerate(_chunk_groups(ka2, 2)):
                            nc.gpsimd.dma_gather(
                                g[:, a0:a1, :], h1all_a[:],
                                idx1_sb[:, OFF16[b] + a0 * 8 : OFF16[b] + a1 * 8],
                                num_idxs=(a1 - a0) * 128,
                                num_idxs_reg=(a1 - a0) * 128,
                                elem_size=H2, single_packet=False,
                                queue_num=qi,
                            )
                        for qi, (a0, a1) in enumerate(_chunk_groups(kb - ka2, 2)):
                            nc.gpsimd.dma_gather(
                                g[:, ka2 + a0 : ka2 + a1, :], h1all_b[:],
                                idx1_sb[:, OFF16[b] + (ka2 + a0) * 8 : OFF16[b] + (ka2 + a1) * 8],
                                num_idxs=(a1 - a0) * 128,
                                num_idxs_reg=(a1 - a0) * 128,
                                elem_size=H2, single_packet=False,
                                queue_num=2 + qi,
                            )
                        agg = psAgg2.tile([128, H2], f32, name=f"agg2_{b}", tag="agg2")
                        for i in range(kb // 2):
                            nc.tensor.matmul(
                                agg[:],
                                st_all[:, CI0[b] + 2 * i : CI0[b] + 2 * i + 2, :],
                                g[:, 2 * i : 2 * i + 2, :],
                                start=(i == 0),
                                stop=(i == kb // 2 - 1) and not has_bias,
                                perf_mode=DR,
                            )
                        if has_bias:
                            nc.tensor.matmul(
                                agg[:], ones1[:], b2_sb[:],
                                start=False, stop=True, skip_group_check=True,
                            )
                        x2b = x2p.tile([128, H2], b16, name=f"x2_{b}", tag="x2")
                        x2bs[b] = x2b
                        nc.vector.tensor_scalar_max(x2b[:], agg[:], 0.0)
                      if bb >= LAG:
                        b = bb - LAG
                        tb = TB[b]
                        sl = slice(TB_OFF[b], TB_OFF[b + 1])
                        x2b = x2bs.pop(b)
                        for jj in range(njj):
                            tr = psTr2.tile([128, 128], b16, name=f"tr2_{b}_{jj}", tag="tr2")
                            nc.tensor.transpose(
                                tr[:, :tb],
                                x2b[:tb, jj * 128 : (jj + 1) * 128],
                                ident[:tb, :tb],
                            )
                            nc.vector.tensor_copy(x2T[jj][:, sl], tr[:, :tb])
                        po = psO.tile([128, DO], f32, name=f"pso_{b}", tag="pso")
                        for jj in range(njj):
                            nc.tensor.matmul(
                                po[:tb, :], x2T[jj][:, sl],
                                w3t[:, jj * DO : (jj + 1) * DO],
                                start=(jj == 0),
                                stop=(jj == njj - 1) and not has_bias,
                            )
                        if has_bias:
                            nc.tensor.matmul(
                                po[:tb, :], ones1[:, :tb], b3_sb[:],
                                start=False, stop=True, skip_group_check=True,
                            )
                        oo = outp.tile([128, DO], f32, name=f"oo_{b}", tag="oo")
                        nc.vector.tensor_scalar_max(oo[:tb, :], po[:tb, :], 0.0)
                        nc.sync.dma_start(out_d.ap()[sl, :], oo[:tb, :])

    nc.compile()
    return nc


def kernel_with_results(features, src, dst, W1, b1, W2, b2, W3, b3, trace=False):
    static, in_maps = _host_prep(features, src, dst, W1, b1, W2, b2, W3, b3)
    nc = _build_program(static)
    res = bass_utils.run_bass_kernel_spmd(
        nc, in_maps, core_ids=list(range(NCORES)), trace=trace
    )
    out = np.concatenate([res.results[c]["out"] for c in range(NCORES)], axis=0)
    return out.astype(np.float32), res


def kernel(features, src, dst, W1, b1, W2, b2, W3, b3):
    out, _ = kernel_with_results(features, src, dst, W1, b1, W2, b2, W3, b3)
    return out


# revision 21
# speedup vs baseline: 1.0813x; 1.0609x over previous
"""GCN message-passing kernel for 8 trn2 NeuronCores (Bass/Tile) — v2.

Math (reference):
  x1 = relu(segsum(feat) @ W1 + b1)
  x2 = relu(segsum(x1) @ W2 + b2)
  out = relu(x2 @ W3 + b3)
where segsum(X)[i] = sum_{e: dst[e]=i} X[src[e]].

Reorder: segsum(X) @ W == segsum(X @ W):
  h0 = feat @ W1            (token-major matmul, bf16)
  x1 = relu(segsum(h0)+b1)  (gather h0 rows by src, segment-sum by dst)
  h1 = x1 @ W2
  x2 = relu(segsum(h1)+b2)
  out = relu(x2 @ W3 + b3)

v2 changes vs v1 (971us baseline):
 - h0/h1 stored fp8e4m3: AllGather + gather DMA bytes halved; S tiles fp8
   (integer counts, exact). Aggregation matmuls fp8 DoubleRow (2 chunk pairs
   per PE pass). Phase-A / W2 / W3 matmuls stay bf16 (fp8 there breaks the
   2e-2 rel-err budget; measured 2.05e-2 in numpy sim).
 - AllGather(h0) split by column halves: AG(half0) overlaps phase-A compute
   of half1; AG(half1) overlaps L1 aggregation of half0.
 - AllGather(h1) split by row halves (blocks 0-9 / 10-19) so the first AG
   overlaps the tail of L1/W2; gather row ids for layer 2 are remapped to the
   concatenated [a;b] layout on the host.
 - S selection tiles loaded into SBUF once and reused across both column
   halves and both layers (same edge structure).
 - Gathers spread across 2 SWDGE queues.
"""
import numpy as np
import ml_dtypes

import concourse.bass as bass
import concourse.bacc as bacc
import concourse.tile as tile
import concourse.mybir as mybir
from concourse import bass_utils

bf16 = ml_dtypes.bfloat16
f8e4 = ml_dtypes.float8_e4m3

NCORES = 8
N_NODES = 20000
N_EDGES = 200000
D_IN = 1433
KF = 1536           # padded feature dim (12 x 128)
H1 = 1024           # padded hidden1 (real 1000)
H2 = 512            # padded hidden2 (real 500)
DO = 7
R = N_NODES // NCORES          # 2500 rows per core
TB = [128] * 19 + [68]         # token/dst blocks per core (sum = 2500)
NB = len(TB)
TB_OFF = np.concatenate([[0], np.cumsum(TB)]).astype(int)
NBA = 10                        # blocks in first h1 AllGather row-split
RS_A = int(TB_OFF[NBA])         # 1280 rows
RS_B = R - RS_A                 # 1220 rows


def _chunk_groups(kb, nq=4):
    """Split kb chunks into up to nq contiguous groups of near-equal size."""
    nq = min(nq, kb)
    base = kb // nq
    rem = kb % nq
    sizes = [base + (1 if i < rem else 0) for i in range(nq)]
    out = []
    a = 0
    for s in sizes:
        out.append((a, a + s))
        a += s
    return out


def _wrap_idx(ids, npad):
    """int16 wrapped gather-idx layout: [16, npad/16] tiled to 128 partitions."""
    pad = np.zeros(npad, np.int64)
    pad[: len(ids)] = ids
    wrapped = pad.reshape(-1, 16).T.astype(np.int16)  # [16, npad/16]
    return np.tile(wrapped, (8, 1))                   # [128, npad/16]


def _host_prep(features, src, dst, W1, b1, W2, b2, W3, b3):
    """Build per-core staged arrays (all sharding/sorting/padding on host)."""
    feat = np.asarray(features, np.float32)
    src = np.asarray(src).astype(np.int64)
    dst = np.asarray(dst).astype(np.int64)

    featT = np.zeros((KF, N_NODES), np.float32)
    featT[:D_IN, :] = feat.T
    featT = featT.astype(bf16)

    W1p = np.zeros((KF, H1), np.float32)
    W1p[:D_IN, : W1.shape[1]] = W1
    W1p = W1p.astype(bf16)
    W2p = np.zeros((H1, H2), np.float32)
    W2p[: W2.shape[0], : W2.shape[1]] = W2
    W2p = W2p.astype(bf16)
    # W3 host-swizzled to [128, 4*DO]: W3sw[p, j*DO:(j+1)*DO] = W3p[j*128+p, :]
    W3p = np.zeros((H2, DO), np.float32)
    W3p[: W3.shape[0], :] = W3
    W3sw = np.zeros((128, (H2 // 128) * DO), np.float32)
    for j in range(H2 // 128):
        W3sw[:, j * DO : (j + 1) * DO] = W3p[j * 128 : (j + 1) * 128, :]
    W3sw = W3sw.astype(bf16)

    b1p = np.zeros((1, H1), np.float32)
    b1p[0, : b1.shape[0]] = b1
    b2p = np.zeros((1, H2), np.float32)
    b2p[0, : b2.shape[0]] = b2
    b3p = np.zeros((1, DO), np.float32)
    b3p[0, : b3.shape[0]] = b3
    has_bias = bool(np.any(b1p) or np.any(b2p) or np.any(b3p))

    ident = np.eye(128, dtype=bf16)

    # ---- edge prep: partition by dst owner, sort by dst, chunk per dst-block
    owner = dst // R
    edge_src = [[] for _ in range(NCORES)]
    for c in range(NCORES):
        sel = np.nonzero(owner == c)[0]
        d_loc = dst[sel] - c * R
        order = np.argsort(d_loc, kind="stable")
        sel = sel[order]
        d_loc = d_loc[order]
        s_glob = src[sel]
        blk_of = np.searchsorted(TB_OFF[1:], d_loc, side="right")
        per_blk = []
        for b in range(NB):
            m = blk_of == b
            uniq, inv = np.unique(s_glob[m], return_inverse=True)
            per_blk.append((uniq, inv, d_loc[m] - TB_OFF[b]))
        edge_src[c] = per_blk

    # split each block's unique srcs by source-row region (r < RS_A within the
    # owning core -> region a). Region-wise EVEN chunk counts, uniform across
    # cores (SPMD + DoubleRow pairs). Region a reads h0all_a (row-split AG),
    # region b reads h0all_b.
    def _region_of(ids):
        return (ids % R) < RS_A

    per_cb = [[None] * NB for _ in range(NCORES)]
    Ka_blk = [2] * NB
    Kb_blk = [2] * NB
    for c in range(NCORES):
        for b in range(NB):
            uniq, inv, d_l = edge_src[c][b]
            in_a = _region_of(uniq)
            ua, ub = uniq[in_a], uniq[~in_a]
            per_cb[c][b] = (ua, ub, uniq, inv, d_l)
            ka = -(-len(ua) // 128)
            kb_ = -(-len(ub) // 128)
            Ka_blk[b] = max(Ka_blk[b], ka + (ka & 1))
            Kb_blk[b] = max(Kb_blk[b], kb_ + (kb_ & 1))
    K_blk = [Ka_blk[b] + Kb_blk[b] for b in range(NB)]
    TC = sum(K_blk)
    CI0 = np.concatenate([[0], np.cumsum(K_blk)]).astype(int)
    OFF16 = np.concatenate([[0], np.cumsum([k * 8 for k in K_blk])]).astype(int)
    TOT16 = int(OFF16[-1])

    # h0 row-split remap: node (c, r): r < RS_A -> h0all_a row c*RS_A + r,
    # else h0all_b row c*RS_B + (r - RS_A)
    def h0_remap(ids):
        c = ids // R
        r = ids - c * R
        return np.where(r < RS_A, c * RS_A + r, c * RS_B + (r - RS_A))

    idx1_per_core = []
    s_sw_per_core = []
    for c in range(NCORES):
        i1 = np.zeros((128, TOT16), np.int16)
        s_f32 = np.zeros((TC, 128, 128), np.float32)
        for b in range(NB):
            ua, ub, uniq, inv, d_l = per_cb[c][b]
            # slot order: region-a srcs in chunks [0, Ka), region-b srcs in
            # chunks [Ka, Ka+Kb). Map each edge's uniq-index to its slot.
            ka, kb_ = Ka_blk[b], Kb_blk[b]
            slot_of_uniq = np.zeros(len(uniq), np.int64)
            in_a = _region_of(uniq)
            slot_of_uniq[in_a] = np.arange(len(ua))
            slot_of_uniq[~in_a] = ka * 128 + np.arange(len(ub))
            slot = slot_of_uniq[inv]
            np.add.at(s_f32, (CI0[b] + slot // 128, slot % 128, d_l), 1.0)
            i1[:, OFF16[b] : OFF16[b] + ka * 8] = _wrap_idx(h0_remap(ua), ka * 128)
            i1[:, OFF16[b] + ka * 8 : OFF16[b + 1]] = _wrap_idx(h0_remap(ub), kb_ * 128)
        idx1_per_core.append(i1)
        s_sw_per_core.append(
            np.ascontiguousarray(s_f32.astype(f8e4).transpose(1, 0, 2))
        )

    static = dict(K_blk=K_blk, Ka_blk=Ka_blk, Kb_blk=Kb_blk, TC=TC, CI0=CI0,
                  OFF16=OFF16, TOT16=TOT16, has_bias=has_bias)
    shared = dict(W1p=W1p, W2p=W2p, W3sw=W3sw, b1p=b1p.astype(bf16),
                  b2p=b2p.astype(bf16), b3p=b3p, ident=ident)
    in_maps = []
    for c in range(NCORES):
        in_maps.append(
            dict(
                featT=np.ascontiguousarray(featT[:, c * R : (c + 1) * R]),
                idx1=idx1_per_core[c],
                s_sw=s_sw_per_core[c],
                **shared,
            )
        )
    return static, in_maps


def _build_program(static):
    K_blk, TC, CI0, OFF16, TOT16 = (
        static["K_blk"], static["TC"], static["CI0"], static["OFF16"], static["TOT16"],
    )
    Ka_blk, Kb_blk = static["Ka_blk"], static["Kb_blk"]
    has_bias = static["has_bias"]
    f32 = mybir.dt.float32
    b16 = mybir.dt.bfloat16
    e4 = mybir.dt.float8e4
    i16 = mybir.dt.int16
    DR = mybir.MatmulPerfMode.DoubleRow

    nc = bacc.Bacc(
        "TRN2", target_bir_lowering=False, debug=False,
        enable_asserts=False, num_devices=NCORES, num_swdge_queues=4,
    )

    featT_d = nc.dram_tensor("featT", [KF, R], b16, kind="ExternalInput")
    W1_d = nc.dram_tensor("W1p", [KF, H1], b16, kind="ExternalInput")
    W2_d = nc.dram_tensor("W2p", [H1, H2], b16, kind="ExternalInput")
    W3_d = nc.dram_tensor("W3sw", [128, (H2 // 128) * DO], b16, kind="ExternalInput")
    b1_d = nc.dram_tensor("b1p", [1, H1], b16, kind="ExternalInput")
    b2_d = nc.dram_tensor("b2p", [1, H2], b16, kind="ExternalInput")
    b3_d = nc.dram_tensor("b3p", [1, DO], f32, kind="ExternalInput")
    id_d = nc.dram_tensor("ident", [128, 128], b16, kind="ExternalInput")
    idx1_d = nc.dram_tensor("idx1", [128, TOT16], i16, kind="ExternalInput")
    ssw_d = nc.dram_tensor("s_sw", [128, TC, 128], e4, kind="ExternalInput")
    out_d = nc.dram_tensor("out", [R, DO], f32, kind="ExternalOutput")

    kmax = max(K_blk)

    with tile.TileContext(nc) as tc:
        qsem = [nc.alloc_semaphore(name=f"swdge_q{q}") for q in range(4)]
        with (
            tc.tile_pool(name="const", bufs=1) as constp,
            tc.tile_pool(name="w2p", bufs=H1 // 128) as w2p,
            tc.tile_pool(name="w3p", bufs=1) as w3p,
            tc.tile_pool(name="dram", bufs=1, space="DRAM") as dram,
        ):
            # ---- constants
            idx1_sb = constp.tile([128, TOT16], i16, tag="idx1")
            nc.sync.dma_start(idx1_sb[:], idx1_d.ap())
            st_all = constp.tile([128, TC, 128], e4, tag="st_all")
            nc.sync.dma_start(st_all[:], ssw_d.ap())
            ident = constp.tile([128, 128], b16, tag="ident")
            nc.sync.dma_start(ident[:], id_d.ap())
            ones1 = constp.tile([1, 128], b16, tag="ones1")
            nc.vector.memset(ones1[:], 1.0)
            b1_sb = constp.tile([1, H1], b16, tag="b1")
            nc.sync.dma_start(b1_sb[:], b1_d.ap())
            b2_sb = constp.tile([1, H2], b16, tag="b2")
            nc.sync.dma_start(b2_sb[:], b2_d.ap())
            b3_sb = constp.tile([1, DO], f32, tag="b3")
            nc.sync.dma_start(b3_sb[:], b3_d.ap())

            # ---- DRAM scratch (fp8 halves of h0; h1 row-split for the AG)
            h0_in_a = dram.tile([RS_A, H1], e4, name="h0ina", tag="h0ina")
            h0_in_b = dram.tile([RS_B, H1], e4, name="h0inb", tag="h0inb")
            h0all_a = dram.tile([NCORES * RS_A, H1], e4, name="h0alla", tag="h0alla", addr_space="Shared")
            h0all_b = dram.tile([NCORES * RS_B, H1], e4, name="h0allb", tag="h0allb", addr_space="Shared")
            h1_in_a = dram.tile([RS_A, H2], e4, name="h1ina", tag="h1ina")
            h1_in_b = dram.tile([RS_B, H2], e4, name="h1inb", tag="h1inb")
            h1all_a = dram.tile([NCORES * RS_A, H2], e4, name="h1alla", tag="h1alla", addr_space="Shared")
            h1all_b = dram.tile([NCORES * RS_B, H2], e4, name="h1allb", tag="h1allb", addr_space="Shared")

            # ================= Phase A: h0 = featT.T @ W1, column halves
            w2c = []
            for j in range(H1 // 128):
                wt = w2p.tile([128, H2], b16, name=f"w2c{j}", tag="w2c")
                nc.sync.dma_start(wt[:], W2_d.ap()[j * 128 : (j + 1) * 128, :])
                w2c.append(wt)
            w3t = w3p.tile([128, (H2 // 128) * DO], b16, tag="w3")
            nc.sync.dma_start(w3t[:], W3_d.ap())

            with (
                tc.tile_pool(name="featp", bufs=KF // 128) as featp,
                tc.tile_pool(name="w1p", bufs=KF // 128) as w1p,
            ):
                featc = []
                w1c = []
                for k in range(KF // 128):
                    ft = featp.tile([128, R], b16, name=f"featc{k}", tag="featc")
                    nc.sync.dma_start(ft[:], featT_d.ap()[k * 128 : (k + 1) * 128, :])
                    featc.append(ft)
                    wt = w1p.tile([128, H1], b16, name=f"w1c{k}", tag="w1c")
                    nc.sync.dma_start(wt[:], W1_d.ap()[k * 128 : (k + 1) * 128, :])
                    w1c.append(wt)
                with (
                    tc.tile_pool(name="h0out", bufs=4) as h0outp,
                    tc.tile_pool(name="psA", bufs=6, space="PSUM") as psA,
                ):
                    nk = KF // 128
                    with nc.named_scope("phaseA"):
                        for t in range(NB):
                            sl = slice(TB_OFF[t], TB_OFF[t + 1])
                            tb = TB[t]
                            ps = [psA.tile([128, 512], f32, name=f"psA_{j}_{t}", tag="psA")
                                  for j in range(2)]
                            for k in range(nk):
                                for j in range(2):
                                    nc.tensor.matmul(
                                        ps[j][:tb, :],
                                        featc[k][:, sl],
                                        w1c[k][:, j * 512 : (j + 1) * 512],
                                        start=(k == 0),
                                        stop=(k == nk - 1),
                                    )
                            o = h0outp.tile([128, H1], e4, name=f"h0o_{t}", tag="h0o")
                            for j in range(2):
                                nc.vector.tensor_copy(
                                    o[:tb, j * 512 : (j + 1) * 512], ps[j][:tb, :])
                            if t < NBA:
                                nc.sync.dma_start(h0_in_a[:][sl, :], o[:tb, :])
                            else:
                                sl_b = slice(TB_OFF[t] - RS_A, TB_OFF[t + 1] - RS_A)
                                nc.sync.dma_start(h0_in_b[:][sl_b, :], o[:tb, :])
                            if t == NBA - 1:
                                nc.gpsimd.collective_compute(
                                    "AllGather", mybir.AluOpType.bypass,
                                    replica_groups=[list(range(NCORES))],
                                    ins=[h0_in_a.opt()], outs=[h0all_a.opt()],
                                )
            # ================= L1 aggregation per half + transpose; W2 -> h1
            with (
                tc.tile_pool(name="gout", bufs=7) as goutp,
                tc.tile_pool(name="x1h", bufs=4) as x1hp,
                tc.tile_pool(name="x1T", bufs=H1 // 128) as x1Tp,
                tc.tile_pool(name="h1o", bufs=3) as h1op,
                tc.tile_pool(name="psAgg", bufs=4, space="PSUM") as psAgg,
                tc.tile_pool(name="psTr", bufs=2, space="PSUM") as psTr,
                tc.tile_pool(name="psH1", bufs=2, space="PSUM") as psH1,
            ):
                x1T = [x1Tp.tile([128, R], b16, name=f"x1T{_j}", tag="x1T") for _j in range(H1 // 128)]

                nj = H1 // 128
                LAG = 2
                x1hs = {}

                def _emit_ga(b, g):
                    ka = Ka_blk[b]
                    for qi, (a0, a1) in enumerate(_chunk_groups(ka, 2)):
                        nc.gpsimd.dma_gather(
                            g[:, a0:a1, :], h0all_a[:],
                            idx1_sb[:, OFF16[b] + a0 * 8 : OFF16[b] + a1 * 8],
                            num_idxs=(a1 - a0) * 128,
                            num_idxs_reg=(a1 - a0) * 128,
                            elem_size=H1, single_packet=False,
                            queue_num=qi,
                        )

                with nc.named_scope("phaseC"):
                    nc.gpsimd.collective_compute(
                        "AllGather", mybir.AluOpType.bypass,
                        replica_groups=[list(range(NCORES))],
                        ins=[h0_in_b.opt()], outs=[h0all_b.opt()],
                    )
                    for bb in range(NB + LAG):
                      if bb < NB:
                        b = bb
                        kb = K_blk[b]
                        sl = slice(TB_OFF[b], TB_OFF[b + 1])
                        g = goutp.tile([128, kmax, H1], e4, name=f"gout_{b}", tag="gout")
                        _emit_ga(b, g)
                        ka = Ka_blk[b]
                        for qi, (a0, a1) in enumerate(_chunk_groups(kb - ka, 2)):
                            nc.gpsimd.dma_gather(
                                g[:, ka + a0 : ka + a1, :], h0all_b[:],
                                idx1_sb[:, OFF16[b] + (ka + a0) * 8 : OFF16[b] + (ka + a1) * 8],
                                num_idxs=(a1 - a0) * 128,
                                num_idxs_reg=(a1 - a0) * 128,
                                elem_size=H1, single_packet=False,
                                queue_num=2 + qi,
                            )
                        aggs = [psAgg.tile([128, 512], f32, name=f"agg_{j}_{b}", tag="agg")
                                for j in range(2)]
                        pair_off = [2 * i for i in range(ka // 2)] + \
                                   [ka + 2 * i for i in range((kb - ka) // 2)]
                        for pi, po_ in enumerate(pair_off):
                            for j in range(2):
                                nc.tensor.matmul(
                                    aggs[j][:],
                                    st_all[:, CI0[b] + po_ : CI0[b] + po_ + 2, :],
                                    g[:, po_ : po_ + 2, j * 512 : (j + 1) * 512],
                                    start=(pi == 0),
                                    stop=(pi == len(pair_off) - 1) and not has_bias,
                                    perf_mode=DR,
                                )
                        if has_bias:
                            for j in range(2):
                                nc.tensor.matmul(
                                    aggs[j][:], ones1[:],
                                    b1_sb[:, j * 512 : (j + 1) * 512],
                                    start=False, stop=True, skip_group_check=True,
                                )
                        x1h = x1hp.tile([128, H1], b16, name=f"x1h_{b}", tag="x1h")
                        x1hs[b] = x1h
                        for j in range(2):
                            nc.vector.tensor_scalar_max(
                                x1h[:, j * 512 : (j + 1) * 512], aggs[j][:], 0.0)
                      if bb >= LAG:
                        b = bb - LAG
                        tb = TB[b]
                        sl = slice(TB_OFF[b], TB_OFF[b + 1])
                        x1h = x1hs.pop(b)
                        for jj in range(8):
                            tr = psTr.tile([128, 128], b16, name=f"tr_{b}_{jj}", tag="tr")
                            nc.tensor.transpose(
                                tr[:, :tb],
                                x1h[:tb, jj * 128 : (jj + 1) * 128],
                                ident[:tb, :tb],
                            )
                            nc.vector.tensor_copy(x1T[jj][:, sl], tr[:, :tb])
                        ph = psH1.tile([128, H2], f32, name=f"psh1_{b}", tag="psh1")
                        for j in range(nj):
                            nc.tensor.matmul(
                                ph[:tb, :], x1T[j][:, sl], w2c[j][:],
                                start=(j == 0), stop=(j == nj - 1),
                            )
                        ho = h1op.tile([128, H2], e4, name=f"h1o_{b}", tag="h1o")
                        nc.vector.tensor_copy(ho[:tb, :], ph[:tb, :])
                        if b < NBA:
                            nc.sync.dma_start(h1_in_a[:][sl, :], ho[:tb, :])
                        else:
                            sl_b2 = slice(TB_OFF[b] - RS_A, TB_OFF[b + 1] - RS_A)
                            nc.sync.dma_start(h1_in_b[:][sl_b2, :], ho[:tb, :])
                        if b == NBA - 1:
                            nc.gpsimd.collective_compute(
                                "AllGather", mybir.AluOpType.bypass,
                                replica_groups=[list(range(NCORES))],
                                ins=[h1_in_a.opt()], outs=[h1all_a.opt()],
                            )
                    nc.gpsimd.collective_compute(
                        "AllGather", mybir.AluOpType.bypass,
                        replica_groups=[list(range(NCORES))],
                        ins=[h1_in_b.opt()], outs=[h1all_b.opt()],
                    )

            # ================= L2 aggregation + relu; W3 -> out
            with (
                tc.tile_pool(name="gout2", bufs=8) as goutp2,
                tc.tile_pool(name="x2p", bufs=6) as x2p,
                tc.tile_pool(name="x2T", bufs=H2 // 128) as x2Tp,
                tc.tile_pool(name="outp", bufs=3) as outp,
                tc.tile_pool(name="psAgg2", bufs=4, space="PSUM") as psAgg2,
                tc.tile_pool(name="psTr2", bufs=2, space="PSUM") as psTr2,
                tc.tile_pool(name="psO", bufs=2, space="PSUM") as psO,
            ):
                x2T = [x2Tp.tile([128, R], b16, name=f"x2T{_j}", tag="x2T") for _j in range(H2 // 128)]

                with nc.named_scope("phaseF"):
                    njj = H2 // 128
                    LAG = 3
                    x2bs = {}
                    for bb in range(NB + LAG):
                      if bb < NB:
                        b = bb
                        kb = K_blk[b]
                        g = goutp2.tile([128, kmax, H2], e4, name=f"gout2_{b}", tag="gout2")
                        ka2 = Ka_blk[b]
                        for qi, (a0, a1) in enumerate(_chunk_groups(ka2, 2)):
                            nc.gpsimd.dma_gather(
                                g[:, a0:a1, :], h1all_a[:],
                                idx1_sb[:, OFF16[b] + a0 * 8 : OFF16[b] + a1 * 8],
                                num_idxs=(a1 - a0) * 128,
                                num_idxs_reg=(a1 - a0) * 128,
                                elem_size=H2, single_packet=False,
                                queue_num=qi,
                            )
                        for qi, (a0, a1) in enumerate(_chunk_groups(kb - ka2, 2)):
                            nc.gpsimd.dma_gather(
                                g[:, ka2 + a0 : ka2 + a1, :], h1all_b[:],
                                idx1_sb[:, OFF16[b] + (ka2 + a0) * 8 : OFF16[b] + (ka2 + a1) * 8],
                                num_idxs=(a1 - a0) * 128,
                                num_idxs_reg=(a1 - a0) * 128,
                                elem_size=H2, single_packet=False,
                                queue_num=2 + qi,
                            )
                        agg = psAgg2.tile([128, H2], f32, name=f"agg2_{b}", tag="agg2")
                        for i in range(kb // 2):
                            nc.tensor.matmul(
                                agg[:],
                                st_all[:, CI0[b] + 2 * i : CI0[b] + 2 * i + 2, :],
                                g[:, 2 * i : 2 * i + 2, :],
                                start=(i == 0),
                                stop=(i == kb // 2 - 1) and not has_bias,
                                perf_mode=DR,
                            )
                        if has_bias:
                            nc.tensor.matmul(
                                agg[:], ones1[:], b2_sb[:],
                                start=False, stop=True, skip_group_check=True,
                            )
                        x2b = x2p.tile([128, H2], b16, name=f"x2_{b}", tag="x2")
                        x2bs[b] = x2b
                        nc.vector.tensor_scalar_max(x2b[:], agg[:], 0.0)
                      if bb >= LAG:
                        b = bb - LAG
                        tb = TB[b]
                        sl = slice(TB_OFF[b], TB_OFF[b + 1])
                        x2b = x2bs.pop(b)
                        for jj in range(njj):
                            tr = psTr2.tile([128, 128], b16, name=f"tr2_{b}_{jj}", tag="tr2")
                            nc.tensor.transpose(
                                tr[:, :tb],
                                x2b[:tb, jj * 128 : (jj + 1) * 128],
                                ident[:tb, :tb],
                            )
                            nc.vector.tensor_copy(x2T[jj][:, sl], tr[:, :tb])
                        po = psO.tile([128, DO], f32, name=f"pso_{b}", tag="pso")
                        for jj in range(njj):
                            nc.tensor.matmul(
                                po[:tb, :], x2T[jj][:, sl],
                                w3t[:, jj * DO : (jj + 1) * DO],
                                start=(jj == 0),
                                stop=(jj == njj - 1) and not has_bias,
                            )
                        if has_bias:
                            nc.tensor.matmul(
                                po[:tb, :], ones1[:, :tb], b3_sb[:],
                                start=False, stop=True, skip_group_check=True,
                            )
                        oo = outp.tile([128, DO], f32, name=f"oo_{b}", tag="oo")
                        nc.vector.tensor_scalar_max(oo[:tb, :], po[:tb, :], 0.0)
                        nc.sync.dma_start(out_d.ap()[sl, :], oo[:tb, :])

    nc.compile()
    return nc


def kernel_with_results(features, src, dst, W1, b1, W2, b2, W3, b3, trace=False):
    static, in_maps = _host_prep(features, src, dst, W1, b1, W2, b2, W3, b3)
    nc = _build_program(static)
    res = bass_utils.run_bass_kernel_spmd(
        nc, in_maps, core_ids=list(range(NCORES)), trace=trace
    )
    out = np.concatenate([res.results[c]["out"] for c in range(NCORES)], axis=0)
    return out.astype(np.float32), res


def kernel(features, src, dst, W1, b1, W2, b2, W3, b3):
    out, _ = kernel_with_results(features, src, dst, W1, b1, W2, b2, W3, b3)
    return out
